# revision 1
# baseline (speedup 1.0000x reference)
"""BitNet attention (GQA + RoPE) on 8 Trainium2 NeuronCores.

Tensor-parallel over heads: core c owns q-heads [4c, 4c+4), kv-head c.
Each core computes q/k/v projections (ternary BitNet weights), RoPE,
attention for its heads, and a row-parallel partial of the Wo
projection; the host sums the 8 partials.

Dtype strategy (measured on HW):
  - projections / AV / Wo matmuls: float32r (~fp32 precision at
    1 cyc/row for moving dims >= 256)
  - scores matmuls: bf16, K=64 row-tiled so the two heads of a pair
    run concurrently in the PE array (55 ns per [64x128x512] matmul)
  - fp32r with row tiling is ~20x slow on HW; bf16 scores cost ~9e-4
    relative error end-to-end.

The attention mask is folded into the V tile: attn = exp(s*qk + m) =
exp(m)*exp(s*qk), so V rows and the denominator-ones column are
pre-scaled by exp(mask) and the EXP activation needs no bias.

Layout notes (per core):
  qT   [128, 2, 2048]  head-pair p: head 2p on partitions 0:64, head
                       2p+1 on 64:128; RoPE applied; bf16.
  kTd  [128, 2048]     kv head duplicated on both partition halves
                       (lhsT of both row-tiled score matmuls); bf16.
  V    [128, 16, 65]   [sk-chunk, 65] f32r; col 64 = exp(mask) so the
                       AV matmul also emits softmax denominators;
                       cols 0:64 scaled by exp(mask)*s_v*s_o.
  aoT  [128, 2, 2048]  normalized attention outputs, o-major, f32r,
                       lhsT of the Wo matmul.
"""

import sys

if "/opt/trn_rl_repo" not in sys.path:
    sys.path.insert(0, "/opt/trn_rl_repo")

import numpy as np

import concourse.bass as bass
from concourse import bacc, mybir
from concourse.bass import ts
from concourse.bass_utils import run_bass_kernel_spmd
from concourse.masks import make_identity
from concourse.tile import TileContext

F32 = mybir.dt.float32
F32R = mybir.dt.float32r
BF16 = mybir.dt.bfloat16

S = 2048
H = 2048
N_HEADS = 32
N_KV = 8
D = 64
NCORES = 8
HPC = N_HEADS // NCORES  # 4 q heads per core
OC = HPC * D  # 256 output dims per core
NB = S // 512  # 4 s-blocks of 512
HC = H // 128  # 16 hidden chunks

LAST_EXEC_NS = None
LAST_TRACE = None
_CACHE = {}


def _ternarize(w):
    w = np.asarray(w, np.float32)
    s = (np.abs(w).mean() + np.float32(1e-6)).astype(np.float32)
    t = np.round(np.clip(w / s, np.float32(-1.0), np.float32(1.0))).astype(np.float32)
    return t, float(s)


def _build_program(s_qk):
    nc = bacc.Bacc("TRN2", target_bir_lowering=False, debug=False, num_devices=NCORES)

    xt = nc.dram_tensor("xt", [NB, 128, HC, 512], F32R, kind="ExternalInput")
    wq = nc.dram_tensor("wq_t", [128, HC, OC], F32R, kind="ExternalInput")
    wkv = nc.dram_tensor("wkv_t", [128, HC, 128], F32R, kind="ExternalInput")
    wo = nc.dram_tensor("wo_t", [128, 2, H], F32R, kind="ExternalInput")
    cos_d = nc.dram_tensor("cos_t", [128, S], F32, kind="ExternalInput")
    sin_d = nc.dram_tensor("sin_t", [128, S], F32, kind="ExternalInput")
    emv_d = nc.dram_tensor("emv_t", [128, HC], F32, kind="ExternalInput")
    em_d = nc.dram_tensor("em_t", [128, HC], F32R, kind="ExternalInput")
    outp = nc.dram_tensor("outp", [S, H], F32, kind="ExternalOutput")

    EXP = mybir.ActivationFunctionType.Exp
    MUL = mybir.AluOpType.mult
    ADD = mybir.AluOpType.add

    with TileContext(nc) as tc:
        with tc.tile_pool(name="persist", bufs=1) as persist:
            qT = persist.tile([128, 2, S], BF16)
            kTd = persist.tile([128, S], BF16)
            V = persist.tile([128, HC, 65], F32R)
            aoT = persist.tile([128, 2, S], F32R)
            for i in range(HC):
                nc.gpsimd.dma_start(V[:, i, 64:65], em_d[:, i : i + 1])

            # ---- Phase 1: projections + RoPE ----
            with (
                tc.tile_pool(name="ph1w", bufs=1) as ph1w,
                tc.tile_pool(name="xtp", bufs=2) as xtp,
                tc.tile_pool(name="ph1t", bufs=3) as ph1t,
            ):
                wq_sb = ph1w.tile([128, HC, OC], F32R)
                wkv_sb = ph1w.tile([128, HC, 128], F32R)
                for c in range(4):
                    nc.gpsimd.dma_start(wq_sb[:, ts(c, 4), :], wq[:, ts(c, 4), :])
                    nc.gpsimd.dma_start(wkv_sb[:, ts(c, 4), :], wkv[:, ts(c, 4), :])
                cos_sb = ph1w.tile([128, S], F32)
                sin_sb = ph1w.tile([128, S], F32)
                for c in range(2):
                    nc.gpsimd.dma_start(cos_sb[:, ts(c, 1024)], cos_d[:, ts(c, 1024)])
                    nc.gpsimd.dma_start(sin_sb[:, ts(c, 1024)], sin_d[:, ts(c, 1024)])
                emv_sb = ph1w.tile([128, HC], F32)
                nc.gpsimd.dma_start(emv_sb[:], emv_d[:])
                ident = ph1w.tile([128, 128], F32)
                make_identity(nc, ident[:])
                vT = ph1w.tile([64, S], F32)

                with (
                    tc.tile_pool(name="ps1", bufs=2, space="PSUM") as ps1,
                    tc.tile_pool(name="psvt", bufs=2, space="PSUM") as psvt,
                ):
                    for b in range(NB):
                        xt_t = xtp.tile([128, HC, 512], F32R, tag="xt")
                        for c4 in range(4):
                            nc.sync.dma_start(
                                xt_t[:, ts(c4, 4), :], xt[b, :, ts(c4, 4), :]
                            )
                        pq0 = ps1.tile([128, 512], F32, tag="q0")
                        pq1 = ps1.tile([128, 512], F32, tag="q1")
                        pkv = ps1.tile([128, 512], F32, tag="kv")
                        for c in range(HC):
                            st, sp = c == 0, c == HC - 1
                            nc.tensor.matmul(
                                pq0[:], wq_sb[:, c, 0:128], xt_t[:, c, :], start=st, stop=sp
                            )
                            nc.tensor.matmul(
                                pq1[:], wq_sb[:, c, 128:256], xt_t[:, c, :], start=st, stop=sp
                            )
                            nc.tensor.matmul(
                                pkv[:], wkv_sb[:, c, :], xt_t[:, c, :], start=st, stop=sp
                            )
                        sb = ts(b, 512)
                        nc.vector.tensor_copy(vT[:, sb], pkv[64:128, :])
                        for i4 in range(4):
                            i = 4 * b + i4
                            pt = psvt.tile([128, 64], F32, tag="vt")
                            nc.tensor.transpose(
                                pt[:], vT[:, ts(i, 128)], ident[0:64, 0:64]
                            )
                            nc.vector.tensor_scalar_mul(
                                V[:, i, 0:64], pt[:], emv_sb[:, i : i + 1]
                            )
                        rotk = ph1t.tile([64, 512], F32, tag="rotk")
                        nc.vector.tensor_copy(rotk[0:32, :], pkv[32:64, :])
                        nc.vector.tensor_copy(rotk[32:64, :], pkv[0:32, :])
                        kc = ph1t.tile([64, 512], F32, tag="kc")
                        nc.vector.tensor_tensor(kc[:], pkv[0:64, :], cos_sb[0:64, sb], MUL)
                        ks = ph1t.tile([64, 512], F32, tag="ks")
                        nc.vector.tensor_tensor(ks[:], rotk[:], sin_sb[0:64, sb], MUL)
                        nc.vector.tensor_tensor(kTd[0:64, sb], kc[:], ks[:], ADD)
                        nc.vector.tensor_tensor(kTd[64:128, sb], kc[:], ks[:], ADD)
                        for p, pq in ((0, pq0), (1, pq1)):
                            rot = ph1t.tile([128, 512], F32, tag="rot")
                            nc.vector.tensor_copy(rot[0:32, :], pq[32:64, :])
                            nc.vector.tensor_copy(rot[32:64, :], pq[0:32, :])
                            nc.vector.tensor_copy(rot[64:96, :], pq[96:128, :])
                            nc.vector.tensor_copy(rot[96:128, :], pq[64:96, :])
                            qc = ph1t.tile([128, 512], F32, tag="qc")
                            nc.vector.tensor_tensor(qc[:], pq[:], cos_sb[:, sb], MUL)
                            qs = ph1t.tile([128, 512], F32, tag="qs")
                            nc.vector.tensor_tensor(qs[:], rot[:], sin_sb[:, sb], MUL)
                            nc.vector.tensor_tensor(qT[:, p, sb], qc[:], qs[:], ADD)

            # ---- Phases 2+3, interleaved: j-blocks outer, Wo pipelined ----
            with (
                tc.tile_pool(name="expp", bufs=6) as expp,
                tc.tile_pool(name="ph2t", bufs=3) as ph2t,
                tc.tile_pool(name="csd", bufs=4, space="DRAM") as csd,
                tc.tile_pool(name="ph3w", bufs=1) as ph3w,
                tc.tile_pool(name="osp", bufs=4) as osp,
                tc.tile_pool(name="pssc", bufs=2, space="PSUM") as pssc,
                tc.tile_pool(name="psav", bufs=1, space="PSUM") as psav,
                tc.tile_pool(name="pso", bufs=2, space="PSUM") as pso_,
            ):
                wo_sb = ph3w.tile([128, 2, H], F32R)
                for k2 in range(2):
                    nc.gpsimd.dma_start(wo_sb[:, k2, :], wo[:, k2, :])

                def emit_wo(j):
                    for jq4 in range(4):
                        jq = 4 * j + jq4
                        for hb in range(4):
                            po = pso_.tile([128, 512], F32, tag="po", name=f"po_{jq}_{hb}")
                            nc.tensor.matmul(
                                po[:], aoT[:, 0, ts(jq, 128)], wo_sb[:, 0, ts(hb, 512)],
                                start=True, stop=False,
                            )
                            nc.tensor.matmul(
                                po[:], aoT[:, 1, ts(jq, 128)], wo_sb[:, 1, ts(hb, 512)],
                                start=False, stop=True,
                            )
                            ob = osp.tile([128, 512], F32, tag="ob", name=f"ob_{jq}_{hb}")
                            nc.vector.tensor_copy(ob[:], po[:])
                            nc.gpsimd.dma_start(outp[ts(jq, 128), ts(hb, 512)], ob[:])

                for j in range(NB):
                    jb = ts(j, 512)
                    for p in range(2):
                        if p == 1 and j > 0:
                            emit_wo(j - 1)
                        pA = psav.tile([65, 512], F32, tag="avA")
                        pB = psav.tile([65, 512], F32, tag="avB")
                        for i in range(HC):
                            psAB = pssc.tile([128, 1024], F32, tag="sAB")
                            nc.tensor.matmul(
                                psAB[:, 0:512], kTd[0:64, ts(i, 128)], qT[0:64, p, jb],
                                start=True, stop=True,
                            )
                            nc.tensor.matmul(
                                psAB[:, 512:1024], kTd[64:128, ts(i, 128)],
                                qT[64:128, p, jb], start=True, stop=True,
                            )
                            e2 = expp.tile([128, 1024], F32R, tag="e2", name=f"e2_{p}_{j}_{i}")
                            nc.scalar.activation(e2[:], psAB[:], EXP, scale=s_qk)
                            st, sp = i == 0, i == HC - 1
                            nc.tensor.matmul(
                                pA[:], V[:, i, :], e2[:, 0:512], start=st, stop=sp
                            )
                            nc.tensor.matmul(
                                pB[:], V[:, i, :], e2[:, 512:1024], start=st, stop=sp
                            )
                        avsA = ph2t.tile([65, 512], F32, tag="avsA")
                        avsB = ph2t.tile([65, 512], F32, tag="avsB")
                        nc.vector.tensor_copy(avsA[:], pA[:])
                        nc.vector.tensor_copy(avsB[:], pB[:])
                        cs = ph2t.tile([33, 512], F32, tag="cs")
                        nc.vector.tensor_copy(cs[0:1, :], avsA[64:65, :])
                        nc.vector.tensor_copy(cs[32:33, :], avsB[64:65, :])
                        rcs = ph2t.tile([33, 512], F32, tag="rcs")
                        nc.vector.reciprocal(rcs[:], cs[:])
                        cs_dram = csd.tile([2, 1, 512], F32, tag="csd")
                        nc.sync.dma_start(cs_dram[0], rcs[0:1, :])
                        nc.sync.dma_start(cs_dram[1], rcs[32:33, :])
                        for h, avs in ((0, avsA), (1, avsB)):
                            cb = ph2t.tile([64, 512], F32, tag="cb")
                            nc.sync.dma_start(
                                cb[:], cs_dram[h].to_broadcast((64, 512))
                            )
                            nc.vector.tensor_tensor(
                                aoT[h * 64 : (h + 1) * 64, p, jb], avs[0:64, :], cb[:], MUL
                            )
                emit_wo(NB - 1)

    nc.compile()
    return nc


def kernel(
    hidden_states,
    attention_mask,
    position_ids,
    wq,
    wk,
    wv,
    wo,
    _trace=False,
):
    global LAST_EXEC_NS, LAST_TRACE
    x = np.asarray(hidden_states, np.float32)[0]  # [S, H]
    mask = np.asarray(attention_mask, np.float32)[0]  # [S]
    pos = np.asarray(position_ids)[0].astype(np.float32)  # [S]

    wq_t, s_q = _ternarize(wq)
    wk_t, s_k = _ternarize(wk)
    wv_t, s_v = _ternarize(wv)
    wo_t, s_o = _ternarize(wo)
    s_qk = float(np.float32(s_q) * np.float32(s_k) / np.float32(8.0))
    s_vo = np.float32(s_v) * np.float32(s_o)

    key = ("v5", s_qk)
    if key not in _CACHE:
        _CACHE.clear()
        _CACHE[key] = _build_program(s_qk)
    nc = _CACHE[key]

    # shared inputs
    xt_host = np.ascontiguousarray(
        x.T.reshape(HC, 128, NB, 512).transpose(2, 1, 0, 3)
    )
    inv = (
        1.0 / (10000.0 ** (np.arange(0, D, 2, dtype=np.float32) / np.float32(D)))
    ).astype(np.float32)
    fr = pos[:, None] * inv[None, :]  # [S, 32]
    emb = np.concatenate([fr, fr], axis=1)  # [S, 64]
    cos64 = np.cos(emb).astype(np.float32)
    sin64 = np.sin(emb).astype(np.float32)
    sin64[:, : D // 2] *= -1.0
    cos128 = np.ascontiguousarray(np.vstack([cos64.T, cos64.T]))  # [128, S]
    sin128 = np.ascontiguousarray(np.vstack([sin64.T, sin64.T]))
    expmask = np.exp(mask).astype(np.float32)  # [S]
    em_r = np.ascontiguousarray(expmask.reshape(HC, 128).T)  # [128, HC]
    emv_r = np.ascontiguousarray((expmask * s_vo).reshape(HC, 128).T)

    in_maps = []
    for c in range(NCORES):
        wq_c = np.ascontiguousarray(
            wq_t[c * OC : (c + 1) * OC, :].T.reshape(HC, 128, OC).transpose(1, 0, 2)
        )
        wk_c = wk_t[c * D : (c + 1) * D, :].T  # [H, 64]
        wv_c = wv_t[c * D : (c + 1) * D, :].T
        wkv_c = np.ascontiguousarray(
            np.concatenate([wk_c, wv_c], axis=1).reshape(HC, 128, 128).transpose(1, 0, 2)
        )
        wo_c = np.ascontiguousarray(
            wo_t[:, c * OC : (c + 1) * OC].T.reshape(2, 128, H).transpose(1, 0, 2)
        )
        in_maps.append(
            {
                "xt": xt_host,
                "wq_t": wq_c,
                "wkv_t": wkv_c,
                "wo_t": wo_c,
                "cos_t": cos128,
                "sin_t": sin128,
                "emv_t": emv_r,
                "em_t": em_r,
            }
        )

    res = run_bass_kernel_spmd(
        nc, in_maps, core_ids=list(range(NCORES)), trace=bool(_trace)
    )
    LAST_EXEC_NS = res.exec_time_ns
    LAST_TRACE = res.instructions_and_trace[1] if res.instructions_and_trace else None

    out = res.results[0]["outp"].astype(np.float32)
    for c in range(1, NCORES):
        out = out + res.results[c]["outp"]
    return out.reshape(1, S, H).astype(np.float32)



# revision 8
# speedup vs baseline: 1.0212x; 1.0212x over previous
"""BitNet attention (GQA + RoPE) on 8 Trainium2 NeuronCores.

Tensor-parallel over heads: core c owns q-heads [4c, 4c+4), kv-head c.
Each core computes q/k/v projections (ternary BitNet weights), RoPE,
attention for its heads, and a row-parallel partial of the Wo
projection; the host sums the 8 fp16 partials in fp32.

v6 dtype/schedule strategy (from trace analysis of the f32r baseline):
  - ALL matmuls fp16: f32r runs at half PE clock on HW
    (fp32_mode=HIGH ~840ns per 512-row pair vs fp16 ~426ns); ternary
    weights are exact in fp16, activations lose ~0.05% (fp16 mantissa),
    far better than the bf16 baseline's 0.4%.
  - All DMA traffic fp16 (x, weights, rope tables, output partials):
    halves HBM bytes vs f32.
  - kv projections for all 4 sequence blocks run first, then q: keys/
    values are the global dependency of phase 2.
  - RoPE rotate-half runs on the PE (multiply by a block-swap
    permutation, sign folded into the sin table) and the PSUM->SBUF
    fp16 copies run on the Scalar engine, which is idle in phase 1 —
    the vector engine was the phase-1 bottleneck in the baseline.
  - Phase 2 is Scalar-engine-bound (exp of 16.8M scores at ~1113ns per
    [128,1024] ACTIVATE). Score pairs are row-tiled (K=64 halves of
    the PE array run concurrently); AV keeps the denominator as V's
    65th column; Wo matmuls of block j-1 are spread one per chunk so
    the PE never bursts and starves the EXP pipeline.
  - softmax reciprocal via reciprocal_approx_fast (~5x faster than the
    8-cyc/elem iterative divide), broadcast via a DRAM round-trip.

Layout notes (per core):
  qT   [128, 2, 2048]  head-pair p: head 2p on partitions 0:64, head
                       2p+1 on 64:128; RoPE applied; fp16.
  kTd  [128, 2048]     kv head duplicated on both partition halves
                       (lhsT of the two row-tiled score matmuls); fp16.
  V    [128, 16, 65]   [sk-chunk, 65] fp16; col 64 = exp(mask) so the
                       AV matmul also emits softmax denominators;
                       cols 0:64 scaled by exp(mask)*s_v*s_o.
  aoT  [128, 2, 2048]  normalized attention outputs, o-major, fp16,
                       lhsT of the Wo matmul.
"""

import sys

if "/opt/trn_rl_repo" not in sys.path:
    sys.path.insert(0, "/opt/trn_rl_repo")

import numpy as np

import concourse.bass as bass
from concourse import bacc, mybir
from concourse.bass import ts
from concourse.bass_utils import run_bass_kernel_spmd
from concourse.masks import make_identity
from concourse.tile import TileContext

F32 = mybir.dt.float32
F16 = mybir.dt.float16

S = 2048
H = 2048
N_HEADS = 32
N_KV = 8
D = 64
NCORES = 8
HPC = N_HEADS // NCORES  # 4 q heads per core
OC = HPC * D  # 256 q dims per core
NB = S // 512  # 4 s-blocks of 512
HC = H // 128  # 16 hidden chunks

LAST_EXEC_NS = None
LAST_TRACE = None
_CACHE = {}


def _ternarize(w):
    w = np.asarray(w, np.float32)
    s = (np.abs(w).mean() + np.float32(1e-6)).astype(np.float32)
    t = np.round(np.clip(w / s, np.float32(-1.0), np.float32(1.0))).astype(np.float32)
    return t, float(s)


def _build_program(s_qk):
    nc = bacc.Bacc("TRN2", target_bir_lowering=False, debug=False, num_devices=NCORES)

    xt = nc.dram_tensor("xt", [NB, 128, HC, 512], F16, kind="ExternalInput")
    wq = nc.dram_tensor("wq_t", [128, HC, OC], F16, kind="ExternalInput")
    wkv = nc.dram_tensor("wkv_t", [128, HC, 128], F16, kind="ExternalInput")
    wo = nc.dram_tensor("wo_t", [128, 2, H], F16, kind="ExternalInput")
    cos_d = nc.dram_tensor("cos_t", [128, S], F16, kind="ExternalInput")
    sin_d = nc.dram_tensor("sin_t", [128, S], F16, kind="ExternalInput")
    em_d = nc.dram_tensor("em_t", [128, HC], F16, kind="ExternalInput")
    emv_d = nc.dram_tensor("emv_t", [128, HC], F32, kind="ExternalInput")
    prot_d = nc.dram_tensor("prot_t", [128, 128], F16, kind="ExternalInput")
    outp = nc.dram_tensor("outp", [S, H], F16, kind="ExternalOutput")

    EXP = mybir.ActivationFunctionType.Exp
    MUL = mybir.AluOpType.mult
    ADD = mybir.AluOpType.add

    with TileContext(nc) as tc:
        with tc.tile_pool(name="persist", bufs=1) as persist:
            qT = persist.tile([128, 2, S], F16)
            kTd = persist.tile([128, S], F16)
            V = persist.tile([128, HC, 65], F16)
            aoT = persist.tile([128, 2, S], F16)
            xt_sb = persist.tile([128, NB, HC, 512], F16)
            wq_sb = persist.tile([128, HC, OC], F16)
            wkv_sb = persist.tile([128, HC, 128], F16)
            wo_sb = persist.tile([128, 2, H], F16)
            cos_sb = persist.tile([128, S], F16)
            sin_sb = persist.tile([128, S], F16)
            emv_sb = persist.tile([128, HC], F32)
            prot = persist.tile([128, 128], F16)
            ident = persist.tile([64, 64], F16)

            # weight/const loads on the gpsimd queue, earliest-needed first
            nc.gpsimd.dma_start(wkv_sb[:], wkv[:])
            for i in range(HC):
                nc.gpsimd.dma_start(V[:, i, 64:65], em_d[:, i : i + 1])
            nc.gpsimd.dma_start(emv_sb[:], emv_d[:])
            nc.gpsimd.dma_start(prot[:], prot_d[:])
            nc.gpsimd.dma_start(cos_sb[:], cos_d[:])
            nc.gpsimd.dma_start(sin_sb[:], sin_d[:])
            for c in range(2):
                nc.gpsimd.dma_start(wq_sb[:, ts(c, 8), :], wq[:, ts(c, 8), :])
            nc.gpsimd.dma_start(wo_sb[:], wo[:])
            make_identity(nc, ident[:])

            # x loads: blocks 0-1 on the sync queue, 2-3 on the scalar queue
            for b in range(NB):
                eng = nc.sync if b < 2 else nc.scalar
                for c4 in range(4):
                    eng.dma_start(
                        xt_sb[:, b, ts(c4, 4), :], xt[b, :, ts(c4, 4), :]
                    )

            # ---- Phase 1a: k/v projections + k-RoPE + V build ----
            # PE work that depends on a ScalarE copy of block b is deferred
            # until after block b+1's projection matmuls (in-order PE queue).
            with (
                tc.tile_pool(name="pskv", bufs=2, space="PSUM") as pskv,
                tc.tile_pool(name="psr", bufs=2, space="PSUM") as psr,
                tc.tile_pool(name="ph1", bufs=2) as ph1,
            ):
                def kv_tail(b, ksb, vt):
                    sb = ts(b, 512)
                    for i4 in range(4):
                        i = 4 * b + i4
                        pt = psr.tile([128, 64], F16, tag="pt")
                        nc.tensor.transpose(pt[:], vt[:, ts(i4, 128)], ident[:])
                        nc.vector.tensor_scalar_mul(
                            V[:, i, 0:64], pt[:], emv_sb[:, i : i + 1]
                        )
                    rotk = psr.tile([64, 512], F32, tag="rotk")
                    nc.tensor.matmul(
                        rotk[:], prot[0:64, 0:64], ksb[:], start=True, stop=True
                    )
                    kc = ph1.tile([64, 512], F16, tag=f"kc{b % 2}")
                    nc.gpsimd.tensor_tensor(kc[:], ksb[:], cos_sb[0:64, sb], MUL)
                    rks = ph1.tile([64, 512], F16, tag=f"rks{b % 2}")
                    nc.scalar.copy(rks[:], rotk[:])
                    ks = ph1.tile([64, 512], F16, tag=f"ks{b % 2}")
                    nc.vector.tensor_tensor(ks[:], rks[:], sin_sb[0:64, sb], MUL)
                    nc.vector.tensor_tensor(kTd[0:64, sb], kc[:], ks[:], ADD)
                    nc.gpsimd.tensor_copy(kTd[64:128, sb], kTd[0:64, sb])

                pending = None
                for b in range(NB):
                    pkv = pskv.tile([128, 512], F32, tag="kv")
                    for c in range(HC):
                        nc.tensor.matmul(
                            pkv[:], wkv_sb[:, c, :], xt_sb[:, b, c, :],
                            start=c == 0, stop=c == HC - 1,
                        )
                    ksb = ph1.tile([64, 512], F16, tag=f"ksb{b % 2}")
                    nc.scalar.copy(ksb[:], pkv[0:64, :])
                    vt = ph1.tile([64, 512], F16, tag=f"vt{b % 2}")
                    nc.scalar.copy(vt[:], pkv[64:128, :])
                    if pending is not None:
                        kv_tail(*pending)
                    pending = (b, ksb, vt)
                kv_tail(*pending)

            # ---- Phase 1b: q projections + RoPE ----
            with (
                tc.tile_pool(name="psq", bufs=2, space="PSUM") as psq,
                tc.tile_pool(name="ph1q", bufs=2) as ph1q,
            ):
                def q_tail(b, p, qsb):
                    sb = ts(b, 512)
                    rotq = psq.tile([128, 512], F32, tag="rot")
                    nc.tensor.matmul(
                        rotq[:], prot[:], qsb[:], start=True, stop=True
                    )
                    qc = ph1q.tile([128, 512], F16, tag=f"qc{p}")
                    nc.gpsimd.tensor_tensor(qc[:], qsb[:], cos_sb[:, sb], MUL)
                    rqs = ph1q.tile([128, 512], F16, tag=f"rqs{p}")
                    nc.scalar.copy(rqs[:], rotq[:])
                    qs = ph1q.tile([128, 512], F16, tag=f"qs{p}")
                    nc.vector.tensor_tensor(qs[:], rqs[:], sin_sb[:, sb], MUL)
                    nc.vector.tensor_tensor(qT[:, p, sb], qc[:], qs[:], ADD)

                pending = None
                for b in range(NB):
                    for p in range(2):
                        pq = psq.tile([128, 512], F32, tag=f"q{p}")
                        for c in range(HC):
                            nc.tensor.matmul(
                                pq[:], wq_sb[:, c, ts(p, 128)], xt_sb[:, b, c, :],
                                start=c == 0, stop=c == HC - 1,
                            )
                        qsb = ph1q.tile([128, 512], F16, tag=f"qsb{p}")
                        nc.scalar.copy(qsb[:], pq[:])
                        if pending is not None:
                            q_tail(*pending)
                        pending = (b, p, qsb)
                q_tail(*pending)

            # ---- Phase 2+3: attention, Wo of block j-1 spread one matmul
            # per key-chunk so the PE never starves the EXP pipeline ----
            with (
                tc.tile_pool(name="expp", bufs=6) as expp,
                tc.tile_pool(name="ph2t", bufs=3) as ph2t,
                tc.tile_pool(name="csd", bufs=4, space="DRAM") as csd,
                tc.tile_pool(name="osp", bufs=4) as osp,
                tc.tile_pool(name="pssc", bufs=2, space="PSUM") as pssc,
                tc.tile_pool(name="psav", bufs=1, space="PSUM") as psav,
                tc.tile_pool(name="pso", bufs=2, space="PSUM") as pso_,
            ):
                # Wo work queue: list of closures, two PE matmuls per
                # (jq, hb) pair plus the copy+store after the second.
                wo_slots = []

                def queue_wo(j):
                    for jq4 in range(4):
                        jq = 4 * j + jq4
                        for hb in range(4):
                            po = pso_.tile(
                                [128, 512], F32, tag="po", name=f"po_{jq}_{hb}"
                            )
                            ob = osp.tile(
                                [128, 512], F16, tag="ob", name=f"ob_{jq}_{hb}"
                            )

                            def mm0(po=po, jq=jq, hb=hb):
                                nc.tensor.matmul(
                                    po[:], aoT[:, 0, ts(jq, 128)],
                                    wo_sb[:, 0, ts(hb, 512)],
                                    start=True, stop=False,
                                )

                            def mm1(po=po, ob=ob, jq=jq, hb=hb):
                                nc.tensor.matmul(
                                    po[:], aoT[:, 1, ts(jq, 128)],
                                    wo_sb[:, 1, ts(hb, 512)],
                                    start=False, stop=True,
                                )
                                nc.vector.tensor_copy(ob[:], po[:])
                                nc.gpsimd.dma_start(
                                    outp[ts(jq, 128), ts(hb, 512)], ob[:]
                                )

                            wo_slots.append(mm0)
                            wo_slots.append(mm1)

                def emit_slot():
                    if wo_slots:
                        wo_slots.pop(0)()

                for j in range(NB):
                    jb = ts(j, 512)
                    if j > 0:
                        queue_wo(j - 1)
                    for p in range(2):
                        pAB = psav.tile([65, 1024], F32, tag="av")
                        e2s = {}

                        def emit_av(i):
                            e2 = e2s.pop(i)
                            st, sp = i == 0, i == HC - 1
                            nc.tensor.matmul(
                                pAB[:, 0:512], V[:, i, :], e2[:, 0:512],
                                start=st, stop=sp, skip_group_check=True,
                            )
                            nc.tensor.matmul(
                                pAB[:, 512:1024], V[:, i, :], e2[:, 512:1024],
                                start=st, stop=sp, skip_group_check=True,
                            )

                        for i in range(HC):
                            psAB = pssc.tile([128, 1024], F32, tag="sAB")
                            nc.tensor.matmul(
                                psAB[:, 0:512], kTd[0:64, ts(i, 128)],
                                qT[0:64, p, jb], start=True, stop=True,
                                skip_group_check=True,
                            )
                            nc.tensor.matmul(
                                psAB[:, 512:1024], kTd[64:128, ts(i, 128)],
                                qT[64:128, p, jb], start=True, stop=True,
                                skip_group_check=True,
                            )
                            e2 = expp.tile(
                                [128, 1024], F16, tag="e2", name=f"e2_{j}_{p}_{i}"
                            )
                            nc.scalar.activation(e2[:], psAB[:], EXP, scale=s_qk)
                            e2s[i] = e2
                            if i >= 2:
                                emit_av(i - 2)
                            emit_slot()
                        emit_av(HC - 2)
                        emit_av(HC - 1)
                        # normalize: aoT = pAB[0:64] * (1/denominator).
                        # reciprocal cost is free-size * 8 cyc regardless of
                        # partitions, so pack both heads' denominator rows on
                        # different partitions of one [33,512] tile.
                        cs = ph2t.tile([33, 512], F32, tag="cs")
                        nc.vector.tensor_copy(cs[0:1, :], pAB[64:65, 0:512])
                        nc.vector.tensor_copy(cs[32:33, :], pAB[64:65, 512:1024])
                        rcs = ph2t.tile([33, 512], F32, tag="rcs")
                        nc.vector.reciprocal(rcs[:], cs[:])
                        cs_dram = csd.tile([2, 1, 512], F32, tag="csd")
                        nc.sync.dma_start(cs_dram[0], rcs[0:1, :])
                        nc.sync.dma_start(cs_dram[1], rcs[32:33, :])
                        cb = ph2t.tile([64, 1024], F32, tag="cb")
                        nc.sync.dma_start(
                            cb[:, 0:512], cs_dram[0].to_broadcast((64, 512))
                        )
                        nc.sync.dma_start(
                            cb[:, 512:1024], cs_dram[1].to_broadcast((64, 512))
                        )
                        nc.vector.tensor_tensor(
                            aoT[0:64, p, jb], pAB[0:64, 0:512], cb[:, 0:512], MUL
                        )
                        nc.vector.tensor_tensor(
                            aoT[64:128, p, jb], pAB[0:64, 512:1024],
                            cb[:, 512:1024], MUL
                        )
                # drain remaining Wo work (block NB-1 and any leftovers)
                while wo_slots:
                    emit_slot()
                queue_wo(NB - 1)
                while wo_slots:
                    emit_slot()

    nc.compile()
    return nc


def kernel(
    hidden_states,
    attention_mask,
    position_ids,
    wq,
    wk,
    wv,
    wo,
    _trace=False,
):
    global LAST_EXEC_NS, LAST_TRACE
    x = np.asarray(hidden_states, np.float32)[0]  # [S, H]
    mask = np.asarray(attention_mask, np.float32)[0]  # [S]
    pos = np.asarray(position_ids)[0].astype(np.float32)  # [S]

    wq_t, s_q = _ternarize(wq)
    wk_t, s_k = _ternarize(wk)
    wv_t, s_v = _ternarize(wv)
    wo_t, s_o = _ternarize(wo)
    s_qk = float(np.float32(s_q) * np.float32(s_k) / np.float32(8.0))
    s_vo = np.float32(s_v) * np.float32(s_o)

    key = ("v6", s_qk)
    if key not in _CACHE:
        _CACHE.clear()
        _CACHE[key] = _build_program(s_qk)
    nc = _CACHE[key]

    # shared inputs
    xt_host = np.ascontiguousarray(
        x.T.reshape(HC, 128, NB, 512).transpose(2, 1, 0, 3)
    ).astype(np.float16)
    inv = (
        1.0 / (10000.0 ** (np.arange(0, D, 2, dtype=np.float32) / np.float32(D)))
    ).astype(np.float32)
    fr = pos[:, None] * inv[None, :]  # [S, 32]
    emb = np.concatenate([fr, fr], axis=1)  # [S, 64]
    cos64 = np.cos(emb).astype(np.float32)
    sin64 = np.sin(emb).astype(np.float32)
    sin64[:, : D // 2] *= -1.0
    cos128 = np.ascontiguousarray(np.vstack([cos64.T, cos64.T])).astype(np.float16)
    sin128 = np.ascontiguousarray(np.vstack([sin64.T, sin64.T])).astype(np.float16)
    expmask = np.exp(mask).astype(np.float32)  # [S]
    em_r = np.ascontiguousarray(expmask.reshape(HC, 128).T).astype(np.float16)
    emv_r = np.ascontiguousarray(
        (expmask * s_vo).reshape(HC, 128).T
    ).astype(np.float32)
    # rotate-half block-swap permutation (lhsT), sign lives in sin128
    i32 = np.eye(32, dtype=np.float16)
    z32 = np.zeros((32, 32), dtype=np.float16)
    p64 = np.block([[z32, i32], [i32, z32]]).astype(np.float16)
    prot_host = np.zeros((128, 128), dtype=np.float16)
    prot_host[0:64, 0:64] = p64
    prot_host[64:128, 64:128] = p64

    in_maps = []
    for c in range(NCORES):
        wq_c = np.ascontiguousarray(
            wq_t[c * OC : (c + 1) * OC, :].T.reshape(HC, 128, OC).transpose(1, 0, 2)
        ).astype(np.float16)
        wk_c = wk_t[c * D : (c + 1) * D, :].T  # [H, 64]
        wv_c = wv_t[c * D : (c + 1) * D, :].T
        wkv_c = np.ascontiguousarray(
            np.concatenate([wk_c, wv_c], axis=1).reshape(HC, 128, 128).transpose(1, 0, 2)
        ).astype(np.float16)
        wo_c = np.ascontiguousarray(
            wo_t[:, c * OC : (c + 1) * OC].T.reshape(2, 128, H).transpose(1, 0, 2)
        ).astype(np.float16)
        in_maps.append(
            {
                "xt": xt_host,
                "wq_t": wq_c,
                "wkv_t": wkv_c,
                "wo_t": wo_c,
                "cos_t": cos128,
                "sin_t": sin128,
                "em_t": em_r,
                "emv_t": emv_r,
                "prot_t": prot_host,
            }
        )

    res = run_bass_kernel_spmd(
        nc, in_maps, core_ids=list(range(NCORES)), trace=bool(_trace)
    )
    LAST_EXEC_NS = res.exec_time_ns
    LAST_TRACE = res.instructions_and_trace[1] if res.instructions_and_trace else None
    globals()["LAST_RES"] = res

    out = res.results[0]["outp"].astype(np.float32)
    for c in range(1, NCORES):
        out = out + res.results[c]["outp"].astype(np.float32)
    return out.reshape(1, S, H).astype(np.float32)


# revision 12
# speedup vs baseline: 1.1670x; 1.1428x over previous
"""BitNet attention (GQA + RoPE) on 8 Trainium2 NeuronCores.

Tensor-parallel over heads: core c owns q-heads [4c, 4c+4), kv-head c.
Each core computes q/k/v projections (ternary BitNet weights), RoPE,
attention for its heads, and a row-parallel partial of the Wo
projection; the host sums the 8 fp16 partials in fp32.

Schedule/dtype strategy (v7, from trace analysis):
  - ALL matmuls fp16: f32r runs at half PE clock on HW; ternary weights
    are exact in fp16, activations lose ~0.05%.
  - All DMA traffic fp16; wv is pre-scaled by s_v*s_o on the host.
  - The attention mask enters as the EXP activation's per-partition
    bias (exp(s*qk + mask_k)), so V needs no mask scaling and is built
    with DMA transposes (no PE transposes, no PSUM scratch).
  - One PSUM layout for the whole kernel (pssc 4 banks / psav 2 /
    pso 2): phase-1 projections accumulate into pssc tiles, RoPE
    rotations into pso tiles, so no bank-reuse stall or PE-clock reset
    (HAM drops the PE to 1.2 GHz after ~3.4us of idle).
  - Phase 1 pipelines kv(b)+q(b) per 512-token block; block 3's q
    projection is deferred into phase 2's per-chunk slack slots.
  - Phase 2 is ScalarE-bound (exp of 16.8M scores, ~1.1us per
    [128,1024] ACTIVATE). Per chunk the PE does one row-tiled score
    pair + one AV pair (+1 slack matmul slot), staying under the EXP
    period so the EXP queue never starves.
  - Normalization is deferred off the critical path: pAB is copied to
    SBUF (uoT) and the reciprocal/broadcast/multiply chain runs behind
    the next block, with the final multiply on the Pool engine.
  - Wo matmuls of block j-1 fill the phase-2 slack slots one at a
    time; output stores go on the Sync DMA queue.

Layout notes (per core):
  qT   [128, 2, 2048]  head-pair p: head 2p on partitions 0:64, head
                       2p+1 on 64:128; RoPE applied; fp16.
  kTd  [128, 2048]     kv head duplicated on both partition halves
                       (lhsT of the two row-tiled score matmuls); fp16.
  V    [128, 16, 65]   [sk-chunk, 65] fp16; col 64 = 1.0 so the AV
                       matmul also emits softmax denominators.
  uoT/aoT [128, 2, 2048] un/normalized attention outputs, o-major,
                       fp16; aoT is lhsT of the Wo matmul.
"""

import sys

if "/opt/trn_rl_repo" not in sys.path:
    sys.path.insert(0, "/opt/trn_rl_repo")

import numpy as np

import concourse.bass as bass
from concourse import bacc, mybir
from concourse.bass import ts
from concourse.bass_utils import run_bass_kernel_spmd
from concourse.tile import TileContext

F32 = mybir.dt.float32
F16 = mybir.dt.float16

S = 2048
H = 2048
N_HEADS = 32
N_KV = 8
D = 64
NCORES = 8
HPC = N_HEADS // NCORES  # 4 q heads per core
OC = HPC * D  # 256 q dims per core
NB = S // 512  # 4 s-blocks of 512
HC = H // 128  # 16 hidden chunks

LAST_EXEC_NS = None
LAST_TRACE = None
LAST_RES = None
_CACHE = {}


def _ternarize(w):
    w = np.asarray(w, np.float32)
    s = (np.abs(w).mean() + np.float32(1e-6)).astype(np.float32)
    t = np.round(np.clip(w / s, np.float32(-1.0), np.float32(1.0))).astype(np.float32)
    return t, float(s)


def _build_program(s_qk):
    nc = bacc.Bacc("TRN2", target_bir_lowering=False, debug=False, num_devices=NCORES)

    xt = nc.dram_tensor("xt", [NB, 128, HC, 512], F16, kind="ExternalInput")
    wq = nc.dram_tensor("wq_t", [128, HC, OC], F16, kind="ExternalInput")
    wkv = nc.dram_tensor("wkv_t", [128, HC, 128], F16, kind="ExternalInput")
    wo = nc.dram_tensor("wo_t", [128, 2, H], F16, kind="ExternalInput")
    cos_d = nc.dram_tensor("cos_t", [128, S], F16, kind="ExternalInput")
    sin_d = nc.dram_tensor("sin_t", [128, S], F16, kind="ExternalInput")
    mask_d = nc.dram_tensor("mask_t", [128, HC], F32, kind="ExternalInput")
    prot_d = nc.dram_tensor("prot_t", [128, 128], F16, kind="ExternalInput")
    outp = nc.dram_tensor("outp", [S, H], F16, kind="ExternalOutput")

    EXP = mybir.ActivationFunctionType.Exp
    MUL = mybir.AluOpType.mult
    ADD = mybir.AluOpType.add

    with TileContext(nc) as tc:
        with tc.tile_pool(name="persist", bufs=1) as persist:
            qT = persist.tile([128, 2, S], F16)
            kTd = persist.tile([128, S], F16)
            V = persist.tile([128, HC, 65], F16)
            uoT = persist.tile([128, 2, S], F16)
            aoT = persist.tile([128, 2, S], F16)
            xt_sb = persist.tile([128, NB, HC, 512], F16)
            wq_sb = persist.tile([128, HC, OC], F16)
            wkv_sb = persist.tile([128, HC, 128], F16)
            wo_sb = persist.tile([128, 2, H], F16)
            cos_sb = persist.tile([128, S], F16)
            sin_sb = persist.tile([128, S], F16)
            mask_sb = persist.tile([128, HC], F32)
            prot = persist.tile([128, 128], F16)

            nc.gpsimd.memset(V[:, :, 64:65], 1.0)
            # weight/const loads on the gpsimd queue, earliest-needed first
            nc.gpsimd.dma_start(wkv_sb[:], wkv[:])
            nc.gpsimd.dma_start(prot[:], prot_d[:])
            nc.gpsimd.dma_start(cos_sb[:], cos_d[:])
            nc.gpsimd.dma_start(sin_sb[:], sin_d[:])
            nc.gpsimd.dma_start(mask_sb[:], mask_d[:])
            for c in range(2):
                nc.gpsimd.dma_start(wq_sb[:, ts(c, 8), :], wq[:, ts(c, 8), :])
            nc.gpsimd.dma_start(wo_sb[:], wo[:])

            # x loads: blocks 0-1 on the sync queue, 2-3 on the scalar queue
            for b in range(NB):
                eng = nc.sync if b < 2 else nc.scalar
                for c4 in range(4):
                    eng.dma_start(
                        xt_sb[:, b, ts(c4, 4), :], xt[b, :, ts(c4, 4), :]
                    )

            with (
                tc.tile_pool(name="ph1", bufs=2) as ph1,
                tc.tile_pool(name="expp", bufs=6) as expp,
                tc.tile_pool(name="ph2t", bufs=3) as ph2t,
                tc.tile_pool(name="csd", bufs=4, space="DRAM") as csd,
                tc.tile_pool(name="osp", bufs=4) as osp,
                tc.tile_pool(name="pssc", bufs=2, space="PSUM") as pssc,
                tc.tile_pool(name="psav", bufs=1, space="PSUM") as psav,
                tc.tile_pool(name="pso", bufs=2, space="PSUM") as pso_,
            ):
                # ---------- phase-1 emission helpers ----------
                def emit_kv(b):
                    pkv = pssc.tile([128, 1024], F32, tag="sAB")
                    for c in range(HC):
                        nc.tensor.matmul(
                            pkv[:, 0:512], wkv_sb[:, c, :], xt_sb[:, b, c, :],
                            start=c == 0, stop=c == HC - 1,
                            skip_group_check=True,
                        )
                    ksb = ph1.tile([64, 512], F16, tag="ksb")
                    nc.scalar.copy(ksb[:], pkv[0:64, 0:512])
                    vt = ph1.tile([64, 512], F16, tag="vt")
                    nc.scalar.copy(vt[:], pkv[64:128, 0:512])
                    return ksb, vt

                def emit_kv_tail(b, ksb, vt):
                    sb = ts(b, 512)
                    for i4 in range(4):
                        vtt = ph1.tile([128, 64], F16, tag=f"vtt{i4 % 2}")
                        nc.sync.dma_start_transpose(vtt[:], vt[:, ts(i4, 128)])
                        nc.gpsimd.tensor_copy(V[:, 4 * b + i4, 0:64], vtt[:])
                    rotk = pso_.tile([128, 512], F32, tag="po")
                    nc.tensor.matmul(
                        rotk[0:64, :], prot[0:64, 0:64], ksb[:],
                        start=True, stop=True,
                    )
                    kc = ph1.tile([64, 512], F16, tag="kc")
                    nc.gpsimd.tensor_tensor(kc[:], ksb[:], cos_sb[0:64, sb], MUL)
                    rks = ph1.tile([64, 512], F16, tag="rks")
                    nc.scalar.copy(rks[:], rotk[0:64, :])
                    ks = ph1.tile([64, 512], F16, tag="ks")
                    nc.vector.tensor_tensor(ks[:], rks[:], sin_sb[0:64, sb], MUL)
                    nc.vector.tensor_tensor(kTd[0:64, sb], kc[:], ks[:], ADD)
                    nc.gpsimd.tensor_copy(kTd[64:128, sb], kTd[0:64, sb])

                def emit_q(b):
                    pq = pssc.tile([128, 1024], F32, tag="sAB")
                    for c in range(HC):
                        nc.tensor.matmul(
                            pq[:, 0:512], wq_sb[:, c, 0:128], xt_sb[:, b, c, :],
                            start=c == 0, stop=c == HC - 1,
                            skip_group_check=True,
                        )
                        nc.tensor.matmul(
                            pq[:, 512:1024], wq_sb[:, c, 128:256],
                            xt_sb[:, b, c, :],
                            start=c == 0, stop=c == HC - 1,
                            skip_group_check=True,
                        )
                    qsb0 = ph1.tile([128, 512], F16, tag="qsb0")
                    nc.scalar.copy(qsb0[:], pq[:, 0:512])
                    qsb1 = ph1.tile([128, 512], F16, tag="qsb1")
                    nc.scalar.copy(qsb1[:], pq[:, 512:1024])
                    return qsb0, qsb1

                def emit_q_rot(b, p, qsb):
                    rotq = pso_.tile([128, 512], F32, tag="po")
                    nc.tensor.matmul(
                        rotq[:], prot[:], qsb[:], start=True, stop=True
                    )
                    return rotq

                def emit_q_tail(b, p, qsb, rotq):
                    sb = ts(b, 512)
                    qc = ph1.tile([128, 512], F16, tag=f"qc{p}")
                    nc.gpsimd.tensor_tensor(qc[:], qsb[:], cos_sb[:, sb], MUL)
                    rqs = ph1.tile([128, 512], F16, tag=f"rqs{p}")
                    nc.scalar.copy(rqs[:], rotq[:])
                    qs = ph1.tile([128, 512], F16, tag=f"qs{p}")
                    nc.vector.tensor_tensor(qs[:], rqs[:], sin_sb[:, sb], MUL)
                    nc.vector.tensor_tensor(qT[:, p, sb], qc[:], qs[:], ADD)

                # ---------- phase 1: blocks 0..2 fully, block 3 kv only ----
                # PE order: kv(b), q(b), rotk(b), rotq(b-1,*) — every rot
                # matmul waits a ScalarE copy that ran during the previous
                # projection, so the PE never stalls.
                prev_q = None
                for b in range(3):
                    ksb, vt = emit_kv(b)
                    qsb0, qsb1 = emit_q(b)
                    emit_kv_tail(b, ksb, vt)
                    if prev_q is not None:
                        pb, p0, p1 = prev_q
                        r0 = emit_q_rot(pb, 0, p0)
                        r1 = emit_q_rot(pb, 1, p1)
                        emit_q_tail(pb, 0, p0, r0)
                        emit_q_tail(pb, 1, p1, r1)
                    prev_q = (b, qsb0, qsb1)
                ksb3, vt3 = emit_kv(3)
                pb, p0, p1 = prev_q
                r0 = emit_q_rot(pb, 0, p0)
                r1 = emit_q_rot(pb, 1, p1)
                emit_q_tail(pb, 0, p0, r0)
                emit_q_tail(pb, 1, p1, r1)
                emit_kv_tail(3, ksb3, vt3)

                # ---------- phase-2 slack slots ----------
                # j=0 carries block 3's q projection; j>=1 carries Wo(j-1).
                slots = []

                def queue_q3():
                    pq = pssc.tile([128, 1024], F32, tag="sAB")
                    for c in range(HC):
                        def mm(c=c, half=0):
                            nc.tensor.matmul(
                                pq[:, ts(half, 512)],
                                wq_sb[:, c, ts(half, 128)],
                                xt_sb[:, 3, c, :],
                                start=c == 0, stop=c == HC - 1,
                                skip_group_check=True,
                            )
                        slots.append(lambda c=c: mm(c, 0))
                        slots.append(lambda c=c: mm(c, 1))

                    def cp():
                        qsb0 = ph1.tile([128, 512], F16, tag="qsb0")
                        nc.scalar.copy(qsb0[:], pq[:, 0:512])
                        qsb1 = ph1.tile([128, 512], F16, tag="qsb1")
                        nc.scalar.copy(qsb1[:], pq[:, 512:1024])
                        qs3.extend([qsb0, qsb1])
                    slots.append(cp)
                    for p in range(2):
                        def rot_and_tail(p=p):
                            r = emit_q_rot(3, p, qs3[p])
                            emit_q_tail(3, p, qs3[p], r)
                        slots.append(rot_and_tail)

                qs3 = []

                def queue_wo(j):
                    for jq4 in range(4):
                        jq = 4 * j + jq4
                        for hb in range(4):
                            po = pso_.tile(
                                [128, 512], F32, tag="po", name=f"po_{jq}_{hb}"
                            )
                            ob = osp.tile(
                                [128, 512], F16, tag="ob", name=f"ob_{jq}_{hb}"
                            )

                            def mm0(po=po, jq=jq, hb=hb):
                                nc.tensor.matmul(
                                    po[:], aoT[:, 0, ts(jq, 128)],
                                    wo_sb[:, 0, ts(hb, 512)],
                                    start=True, stop=False,
                                    skip_group_check=True,
                                )

                            def mm1(po=po, ob=ob, jq=jq, hb=hb):
                                nc.tensor.matmul(
                                    po[:], aoT[:, 1, ts(jq, 128)],
                                    wo_sb[:, 1, ts(hb, 512)],
                                    start=False, stop=True,
                                    skip_group_check=True,
                                )
                                nc.vector.tensor_copy(ob[:], po[:])
                                nc.sync.dma_start(
                                    outp[ts(jq, 128), ts(hb, 512)], ob[:]
                                )

                            slots.append(mm0)
                            slots.append(mm1)

                def emit_slot():
                    if slots:
                        slots.pop(0)()

                # ---------- phase 2 ----------
                queue_q3()
                for j in range(NB):
                    jb = ts(j, 512)
                    if j > 0:
                        queue_wo(j - 1)
                    for p in range(2):
                        pAB = psav.tile([65, 1024], F32, tag="av")
                        e2s = {}

                        def emit_av(i, pAB=pAB, e2s=e2s):
                            e2 = e2s.pop(i)
                            st, sp = i == 0, i == HC - 1
                            nc.tensor.matmul(
                                pAB[:, 0:512], V[:, i, :], e2[:, 0:512],
                                start=st, stop=sp, skip_group_check=True,
                            )
                            nc.tensor.matmul(
                                pAB[:, 512:1024], V[:, i, :], e2[:, 512:1024],
                                start=st, stop=sp, skip_group_check=True,
                            )

                        for i in range(HC):
                            psAB = pssc.tile([128, 1024], F32, tag="sAB")
                            nc.tensor.matmul(
                                psAB[:, 0:512], kTd[0:64, ts(i, 128)],
                                qT[0:64, p, jb], start=True, stop=True,
                                skip_group_check=True,
                            )
                            nc.tensor.matmul(
                                psAB[:, 512:1024], kTd[64:128, ts(i, 128)],
                                qT[64:128, p, jb], start=True, stop=True,
                                skip_group_check=True,
                            )
                            e2 = expp.tile(
                                [128, 1024], F16, tag="e2", name=f"e2_{j}_{p}_{i}"
                            )
                            nc.scalar.activation(
                                e2[:], psAB[:], EXP,
                                bias=mask_sb[:, i : i + 1], scale=s_qk,
                            )
                            e2s[i] = e2
                            if i >= 2:
                                emit_av(i - 2)
                            if not (p == 0 and i < 2):
                                emit_slot()
                        emit_av(HC - 2)
                        emit_av(HC - 1)
                        # stage denominators + raw outputs out of PSUM, then
                        # normalize behind the next block (Pool multiply)
                        cs = ph2t.tile([33, 512], F32, tag="cs")
                        nc.vector.tensor_copy(cs[0:1, :], pAB[64:65, 0:512])
                        nc.vector.tensor_copy(cs[32:33, :], pAB[64:65, 512:1024])
                        nc.vector.tensor_copy(uoT[0:64, p, jb], pAB[0:64, 0:512])
                        nc.vector.tensor_copy(
                            uoT[64:128, p, jb], pAB[0:64, 512:1024]
                        )
                        rcs = ph2t.tile([33, 512], F32, tag="rcs")
                        nc.vector.reciprocal(rcs[:], cs[:])
                        cs_dram = csd.tile([2, 1, 512], F32, tag="csd")
                        nc.sync.dma_start(cs_dram[0], rcs[0:1, :])
                        nc.sync.dma_start(cs_dram[1], rcs[32:33, :])
                        cb = ph2t.tile([128, 512], F32, tag="cb")
                        nc.sync.dma_start(
                            cb[0:64, :], cs_dram[0].to_broadcast((64, 512))
                        )
                        nc.sync.dma_start(
                            cb[64:128, :], cs_dram[1].to_broadcast((64, 512))
                        )
                        nc.gpsimd.tensor_tensor(
                            aoT[0:64, p, jb], uoT[0:64, p, jb], cb[0:64, :], MUL
                        )
                        nc.gpsimd.tensor_tensor(
                            aoT[64:128, p, jb], uoT[64:128, p, jb],
                            cb[64:128, :], MUL
                        )
                    while slots:
                        emit_slot()
                queue_wo(NB - 1)
                while slots:
                    emit_slot()

    nc.compile()
    return nc


def kernel(
    hidden_states,
    attention_mask,
    position_ids,
    wq,
    wk,
    wv,
    wo,
    _trace=False,
):
    global LAST_EXEC_NS, LAST_TRACE, LAST_RES
    x = np.asarray(hidden_states, np.float32)[0]  # [S, H]
    mask = np.asarray(attention_mask, np.float32)[0]  # [S]
    pos = np.asarray(position_ids)[0].astype(np.float32)  # [S]

    wq_t, s_q = _ternarize(wq)
    wk_t, s_k = _ternarize(wk)
    wv_t, s_v = _ternarize(wv)
    wo_t, s_o = _ternarize(wo)
    s_qk = float(np.float32(s_q) * np.float32(s_k) / np.float32(8.0))
    s_vo = np.float32(s_v) * np.float32(s_o)

    key = ("v7", s_qk)
    if key not in _CACHE:
        _CACHE.clear()
        _CACHE[key] = _build_program(s_qk)
    nc = _CACHE[key]

    # shared inputs
    xt_host = np.ascontiguousarray(
        x.T.reshape(HC, 128, NB, 512).transpose(2, 1, 0, 3)
    ).astype(np.float16)
    inv = (
        1.0 / (10000.0 ** (np.arange(0, D, 2, dtype=np.float32) / np.float32(D)))
    ).astype(np.float32)
    fr = pos[:, None] * inv[None, :]  # [S, 32]
    emb = np.concatenate([fr, fr], axis=1)  # [S, 64]
    cos64 = np.cos(emb).astype(np.float32)
    sin64 = np.sin(emb).astype(np.float32)
    sin64[:, : D // 2] *= -1.0
    cos128 = np.ascontiguousarray(np.vstack([cos64.T, cos64.T])).astype(np.float16)
    sin128 = np.ascontiguousarray(np.vstack([sin64.T, sin64.T])).astype(np.float16)
    mask_r = np.ascontiguousarray(mask.reshape(HC, 128).T).astype(np.float32)
    # rotate-half block-swap permutation (lhsT), sign lives in sin128
    i32 = np.eye(32, dtype=np.float16)
    z32 = np.zeros((32, 32), dtype=np.float16)
    p64 = np.block([[z32, i32], [i32, z32]]).astype(np.float16)
    prot_host = np.zeros((128, 128), dtype=np.float16)
    prot_host[0:64, 0:64] = p64
    prot_host[64:128, 64:128] = p64

    in_maps = []
    for c in range(NCORES):
        wq_c = np.ascontiguousarray(
            wq_t[c * OC : (c + 1) * OC, :].T.reshape(HC, 128, OC).transpose(1, 0, 2)
        ).astype(np.float16)
        wk_c = wk_t[c * D : (c + 1) * D, :].T  # [H, 64]
        wv_c = (wv_t[c * D : (c + 1) * D, :] * s_vo).T  # fold s_v*s_o into wv
        wkv_c = np.ascontiguousarray(
            np.concatenate([wk_c, wv_c], axis=1).reshape(HC, 128, 128).transpose(1, 0, 2)
        ).astype(np.float16)
        wo_c = np.ascontiguousarray(
            wo_t[:, c * OC : (c + 1) * OC].T.reshape(2, 128, H).transpose(1, 0, 2)
        ).astype(np.float16)
        in_maps.append(
            {
                "xt": xt_host,
                "wq_t": wq_c,
                "wkv_t": wkv_c,
                "wo_t": wo_c,
                "cos_t": cos128,
                "sin_t": sin128,
                "mask_t": mask_r,
                "prot_t": prot_host,
            }
        )

    res = run_bass_kernel_spmd(
        nc, in_maps, core_ids=list(range(NCORES)), trace=bool(_trace)
    )
    LAST_EXEC_NS = res.exec_time_ns
    LAST_TRACE = res.instructions_and_trace[1] if res.instructions_and_trace else None
    LAST_RES = res

    out = res.results[0]["outp"].astype(np.float32)
    for c in range(1, NCORES):
        out = out + res.results[c]["outp"].astype(np.float32)
    return out.reshape(1, S, H).astype(np.float32)


# revision 24
# speedup vs baseline: 1.2438x; 1.0658x over previous
"""BitNet attention (GQA + RoPE) on 8 Trainium2 NeuronCores.

Tensor-parallel over heads: core c owns q-heads [4c, 4c+4), kv-head c.
Each core computes q/k/v projections (ternary BitNet weights), RoPE,
attention for its heads, and a row-parallel partial of the Wo
projection; the host sums the 8 fp16 partials in fp32.

Schedule/dtype strategy (v7, from trace analysis):
  - ALL matmuls fp16: f32r runs at half PE clock on HW; ternary weights
    are exact in fp16, activations lose ~0.05%.
  - All DMA traffic fp16; wv is pre-scaled by s_v*s_o on the host.
  - The attention mask enters as the EXP activation's per-partition
    bias (exp(s*qk + mask_k)), so V needs no mask scaling and is built
    with DMA transposes (no PE transposes, no PSUM scratch).
  - One PSUM layout for the whole kernel (pssc 4 banks / psav 2 /
    pso 2): phase-1 projections accumulate into pssc tiles, RoPE
    rotations into pso tiles, so no bank-reuse stall or PE-clock reset
    (HAM drops the PE to 1.2 GHz after ~3.4us of idle).
  - Phase 1 pipelines kv(b)+q(b) per 512-token block; block 3's q
    projection is deferred into phase 2's per-chunk slack slots.
  - Phase 2 is ScalarE-bound (exp of 16.8M scores, ~1.1us per
    [128,1024] ACTIVATE). Per chunk the PE does one row-tiled score
    pair + one AV pair (+1 slack matmul slot), staying under the EXP
    period so the EXP queue never starves.
  - Normalization is deferred off the critical path: pAB is copied to
    SBUF (uoT) and the reciprocal/broadcast/multiply chain runs behind
    the next block, with the final multiply on the Pool engine.
  - Wo matmuls of block j-1 fill the phase-2 slack slots one at a
    time; output stores go on the Sync DMA queue.

Layout notes (per core):
  qT   [128, 2, 2048]  head-pair p: head 2p on partitions 0:64, head
                       2p+1 on 64:128; RoPE applied; fp16.
  kTd  [128, 2048]     kv head duplicated on both partition halves
                       (lhsT of the two row-tiled score matmuls); fp16.
  V    [128, 16, 65]   [sk-chunk, 65] fp16; col 64 = 1.0 so the AV
                       matmul also emits softmax denominators.
  uoT/aoT [128, 2, 2048] un/normalized attention outputs, o-major,
                       fp16; aoT is lhsT of the Wo matmul.
"""

import sys

if "/opt/trn_rl_repo" not in sys.path:
    sys.path.insert(0, "/opt/trn_rl_repo")

import numpy as np

import concourse.bass as bass
from concourse import bacc, mybir
from concourse.bass import ts
from concourse.bass_utils import run_bass_kernel_spmd
from concourse.tile import TileContext

F32 = mybir.dt.float32
F16 = mybir.dt.float16

S = 2048
H = 2048
N_HEADS = 32
N_KV = 8
D = 64
NCORES = 8
HPC = N_HEADS // NCORES  # 4 q heads per core
OC = HPC * D  # 256 q dims per core
NB = S // 512  # 4 s-blocks of 512
HC = H // 128  # 16 hidden chunks

LAST_EXEC_NS = None
LAST_TRACE = None
LAST_RES = None
_CACHE = {}


def _ternarize(w):
    w = np.asarray(w, np.float32)
    s = (np.abs(w).mean() + np.float32(1e-6)).astype(np.float32)
    t = np.round(np.clip(w / s, np.float32(-1.0), np.float32(1.0))).astype(np.float32)
    return t, float(s)


def _build_program(s_qk):
    nc = bacc.Bacc("TRN2", target_bir_lowering=False, debug=False, num_devices=NCORES)

    xt = nc.dram_tensor("xt", [NB, 128, HC, 512], F16, kind="ExternalInput")
    wq = nc.dram_tensor("wq_t", [128, HC, OC], F16, kind="ExternalInput")
    wkv = nc.dram_tensor("wkv_t", [128, HC, 128], F16, kind="ExternalInput")
    wo = nc.dram_tensor("wo_t", [128, 2, H], F16, kind="ExternalInput")
    cos_d = nc.dram_tensor("cos_t", [64, S], F16, kind="ExternalInput")
    sin_d = nc.dram_tensor("sin_t", [64, S], F16, kind="ExternalInput")
    mask_d = nc.dram_tensor("mask_t", [128, HC], F32, kind="ExternalInput")
    prot_d = nc.dram_tensor("prot_t", [128, 128], F16, kind="ExternalInput")
    outp = nc.dram_tensor("outp", [S, H], F32, kind="ExternalOutput")

    EXP = mybir.ActivationFunctionType.Exp
    MUL = mybir.AluOpType.mult
    ADD = mybir.AluOpType.add

    with TileContext(nc) as tc:
        with tc.tile_pool(name="persist", bufs=1) as persist:
            qT = persist.tile([128, 2, S], F16)
            kTd = persist.tile([128, S], F16)
            V = persist.tile([128, HC, 65], F16)
            uoT = persist.tile([128, 2, S], F16)
            aoT = persist.tile([128, 2, S], F16)
            xt_sb = persist.tile([128, NB, HC, 512], F16)
            wq_sb = persist.tile([128, HC, OC], F16)
            wkv_sb = persist.tile([128, HC, 128], F16)
            wo_sb = persist.tile([128, 2, H], F16)
            cos_sb = persist.tile([128, S], F16)
            sin_sb = persist.tile([128, S], F16)
            mask_sb = persist.tile([128, HC], F32)
            prot = persist.tile([128, 128], F16)

            nc.gpsimd.memset(V[:, :, 64:65], 1.0)
            # weight/const loads on the gpsimd queue, earliest-needed first
            nc.gpsimd.dma_start(wkv_sb[:], wkv[:])
            nc.gpsimd.dma_start(cos_sb[0:64, :], cos_d[:])
            nc.gpsimd.dma_start(sin_sb[0:64, :], sin_d[:])
            nc.gpsimd.dma_start(prot[:], prot_d[:])
            nc.gpsimd.dma_start(mask_sb[:], mask_d[:])
            for c in range(2):
                nc.gpsimd.dma_start(wq_sb[:, ts(c, 8), :], wq[:, ts(c, 8), :])
            nc.gpsimd.dma_start(wo_sb[:], wo[:])
            # duplicate rope tables onto partitions 64:128 on-device
            nc.gpsimd.tensor_copy(cos_sb[64:128, :], cos_sb[0:64, :])
            nc.gpsimd.tensor_copy(sin_sb[64:128, :], sin_sb[0:64, :])

            # x loads, block-0 first, striped across the sync+scalar queues
            for b in range(NB):
                for c4 in range(4):
                    eng = nc.sync if c4 % 2 == 0 else nc.scalar
                    eng.dma_start(
                        xt_sb[:, b, ts(c4, 4), :], xt[b, :, ts(c4, 4), :]
                    )

            with (
                tc.tile_pool(name="ph1", bufs=2) as ph1,
                tc.tile_pool(name="expp", bufs=6) as expp,
                tc.tile_pool(name="ph2t", bufs=3) as ph2t,
                tc.tile_pool(name="csd", bufs=4, space="DRAM") as csd,
                tc.tile_pool(name="osp", bufs=4) as osp,
                tc.tile_pool(name="pssc", bufs=2, space="PSUM") as pssc,
                tc.tile_pool(name="psav", bufs=1, space="PSUM") as psav,
                tc.tile_pool(name="pso", bufs=2, space="PSUM") as pso_,
            ):
                # ---------- phase-1 emission helpers ----------
                def emit_kv(b):
                    pkv = pssc.tile([128, 1024], F32, tag="sAB")
                    for c in range(HC):
                        nc.tensor.matmul(
                            pkv[:, 0:512], wkv_sb[:, c, :], xt_sb[:, b, c, :],
                            start=c == 0, stop=c == HC - 1,
                            skip_group_check=True,
                        )
                    ksb = ph1.tile([64, 512], F16, tag="ksb")
                    nc.scalar.copy(ksb[:], pkv[0:64, 0:512])
                    vt = ph1.tile([64, 512], F16, tag="vt")
                    nc.scalar.copy(vt[:], pkv[64:128, 0:512])
                    return ksb, vt

                def emit_kv_tail(b, ksb, vt):
                    sb = ts(b, 512)
                    for i4 in range(4):
                        vtt = ph1.tile([128, 64], F16, tag=f"vtt{i4 % 2}")
                        nc.sync.dma_start_transpose(vtt[:], vt[:, ts(i4, 128)])
                        nc.gpsimd.tensor_copy(V[:, 4 * b + i4, 0:64], vtt[:])
                    rotk = pso_.tile([128, 512], F32, tag="po")
                    nc.tensor.matmul(
                        rotk[0:64, :], prot[0:64, 0:64], ksb[:],
                        start=True, stop=True,
                    )
                    kc = ph1.tile([64, 512], F16, tag="kc")
                    nc.gpsimd.tensor_tensor(kc[:], ksb[:], cos_sb[0:64, sb], MUL)
                    rks = ph1.tile([64, 512], F16, tag="rks")
                    nc.scalar.copy(rks[:], rotk[0:64, :])
                    ks = ph1.tile([64, 512], F16, tag="ks")
                    nc.vector.tensor_tensor(ks[:], rks[:], sin_sb[0:64, sb], MUL)
                    nc.vector.tensor_tensor(kTd[0:64, sb], kc[:], ks[:], ADD)
                    nc.gpsimd.tensor_copy(kTd[64:128, sb], kTd[0:64, sb])

                def emit_q(b):
                    pq = pssc.tile([128, 1024], F32, tag="sAB")
                    for c in range(HC):
                        nc.tensor.matmul(
                            pq[:, 0:512], wq_sb[:, c, 0:128], xt_sb[:, b, c, :],
                            start=c == 0, stop=c == HC - 1,
                            skip_group_check=True,
                        )
                        nc.tensor.matmul(
                            pq[:, 512:1024], wq_sb[:, c, 128:256],
                            xt_sb[:, b, c, :],
                            start=c == 0, stop=c == HC - 1,
                            skip_group_check=True,
                        )
                    qsb0 = ph1.tile([128, 512], F16, tag="qsb0")
                    nc.scalar.copy(qsb0[:], pq[:, 0:512])
                    qsb1 = ph1.tile([128, 512], F16, tag="qsb1")
                    nc.scalar.copy(qsb1[:], pq[:, 512:1024])
                    return qsb0, qsb1

                def emit_q_rot(b, p, qsb):
                    rotq = pso_.tile([128, 512], F32, tag="po")
                    nc.tensor.matmul(
                        rotq[:], prot[:], qsb[:], start=True, stop=True
                    )
                    return rotq

                def emit_q_tail(b, p, qsb, rotq):
                    sb = ts(b, 512)
                    qc = ph1.tile([128, 512], F16, tag=f"qc{p}")
                    nc.gpsimd.tensor_tensor(qc[:], qsb[:], cos_sb[:, sb], MUL)
                    rqs = ph1.tile([128, 512], F16, tag=f"rqs{p}")
                    nc.scalar.copy(rqs[:], rotq[:])
                    qs = ph1.tile([128, 512], F16, tag=f"qs{p}")
                    nc.vector.tensor_tensor(qs[:], rqs[:], sin_sb[:, sb], MUL)
                    nc.vector.tensor_tensor(qT[:, p, sb], qc[:], qs[:], ADD)

                # ---------- phase 1: blocks 0..2 fully, block 3 kv only ----
                # PE order: kv(b), q(b), rotk(b), rotq(b-1,*) — every rot
                # matmul waits a ScalarE copy that ran during the previous
                # projection, so the PE never stalls.
                prev_q = None
                for b in range(3):
                    ksb, vt = emit_kv(b)
                    qsb0, qsb1 = emit_q(b)
                    emit_kv_tail(b, ksb, vt)
                    if prev_q is not None:
                        pb, p0, p1 = prev_q
                        r0 = emit_q_rot(pb, 0, p0)
                        r1 = emit_q_rot(pb, 1, p1)
                        emit_q_tail(pb, 0, p0, r0)
                        emit_q_tail(pb, 1, p1, r1)
                    prev_q = (b, qsb0, qsb1)
                ksb3, vt3 = emit_kv(3)
                pb, p0, p1 = prev_q
                r0 = emit_q_rot(pb, 0, p0)
                r1 = emit_q_rot(pb, 1, p1)
                emit_q_tail(pb, 0, p0, r0)
                emit_q_tail(pb, 1, p1, r1)
                emit_kv_tail(3, ksb3, vt3)

                # ---------- phase-2 slack slots ----------
                # j=0 carries block 3's q projection; j>=1 carries Wo(j-1).
                slots = []

                def queue_q3():
                    pq = pssc.tile([128, 1024], F32, tag="sAB")
                    for c in range(HC):
                        def mm(c=c, half=0):
                            nc.tensor.matmul(
                                pq[:, ts(half, 512)],
                                wq_sb[:, c, ts(half, 128)],
                                xt_sb[:, 3, c, :],
                                start=c == 0, stop=c == HC - 1,
                                skip_group_check=True,
                            )
                        slots.append(lambda c=c: mm(c, 0))
                        slots.append(lambda c=c: mm(c, 1))

                    def cp():
                        qsb0 = ph1.tile([128, 512], F16, tag="qsb0")
                        nc.vector.tensor_copy(qsb0[:], pq[:, 0:512])
                        qsb1 = ph1.tile([128, 512], F16, tag="qsb1")
                        nc.vector.tensor_copy(qsb1[:], pq[:, 512:1024])
                        qs3.extend([qsb0, qsb1])
                    slots.append(cp)
                    for p in range(2):
                        def rot_and_tail(p=p):
                            sb = ts(3, 512)
                            qsb = qs3[p]
                            rotq = pso_.tile([128, 512], F32, tag="po")
                            nc.tensor.matmul(
                                rotq[:], prot[:], qsb[:], start=True, stop=True
                            )
                            qc = ph1.tile([128, 512], F16, tag=f"qc{p}")
                            nc.gpsimd.tensor_tensor(
                                qc[:], qsb[:], cos_sb[:, sb], MUL
                            )
                            rqs = ph1.tile([128, 512], F16, tag=f"rqs{p}")
                            nc.vector.tensor_copy(rqs[:], rotq[:])
                            qs = ph1.tile([128, 512], F16, tag=f"qs{p}")
                            nc.vector.tensor_tensor(
                                qs[:], rqs[:], sin_sb[:, sb], MUL
                            )
                            nc.vector.tensor_tensor(
                                qT[:, p, sb], qc[:], qs[:], ADD
                            )
                        slots.append(rot_and_tail)

                qs3 = []

                def queue_wo(j):
                    for jq4 in range(4):
                        jq = 4 * j + jq4
                        for hb in range(4):
                            po = pso_.tile(
                                [128, 512], F32, tag="po", name=f"po_{jq}_{hb}"
                            )

                            def mm0(po=po, jq=jq, hb=hb):
                                nc.tensor.matmul(
                                    po[:], aoT[:, 0, ts(jq, 128)],
                                    wo_sb[:, 0, ts(hb, 512)],
                                    start=True, stop=False,
                                    skip_group_check=True,
                                )

                            def mm1(po=po, jq=jq, hb=hb):
                                nc.tensor.matmul(
                                    po[:], aoT[:, 1, ts(jq, 128)],
                                    wo_sb[:, 1, ts(hb, 512)],
                                    start=False, stop=True,
                                    skip_group_check=True,
                                )
                                ob = osp.tile(
                                    [128, 512], F32, tag="ob", name=f"ob{jq}_{hb}"
                                )
                                nc.vector.tensor_copy(ob[:], po[:])
                                nc.sync.dma_start(
                                    outp[ts(jq, 128), ts(hb, 512)], ob[:]
                                )

                            slots.append(mm0)
                            slots.append(mm1)

                def emit_slot():
                    if slots:
                        slots.pop(0)()

                # ---------- phase 2 ----------
                queue_q3()
                for j in range(NB):
                    jb = ts(j, 512)
                    if j > 0:
                        queue_wo(j - 1)
                    for p in range(2):
                        pAB = psav.tile([65, 1024], F32, tag="av")
                        e2s = {}

                        def emit_av(i, pAB=pAB, e2s=e2s):
                            e2 = e2s.pop(i)
                            st, sp = i == 0, i == HC - 1
                            nc.tensor.matmul(
                                pAB[:, 0:512], V[:, i, :], e2[:, 0:512],
                                start=st, stop=sp, skip_group_check=True,
                            )
                            nc.tensor.matmul(
                                pAB[:, 512:1024], V[:, i, :], e2[:, 512:1024],
                                start=st, stop=sp, skip_group_check=True,
                            )

                        for i in range(HC):
                            psAB = pssc.tile([128, 1024], F32, tag="sAB")
                            nc.tensor.matmul(
                                psAB[:, 0:512], kTd[0:64, ts(i, 128)],
                                qT[0:64, p, jb], start=True, stop=True,
                                skip_group_check=True,
                            )
                            nc.tensor.matmul(
                                psAB[:, 512:1024], kTd[64:128, ts(i, 128)],
                                qT[64:128, p, jb], start=True, stop=True,
                                skip_group_check=True,
                            )
                            e2 = expp.tile(
                                [128, 1024], F16, tag="e2", name=f"e2_{j}_{p}_{i}"
                            )
                            nc.scalar.activation(
                                e2[:], psAB[:], EXP,
                                bias=mask_sb[:, i : i + 1], scale=s_qk,
                            )
                            e2s[i] = e2
                            if i >= 4:
                                emit_av(i - 4)
                            if not (p == 0 and i < 4):
                                emit_slot()
                                left = (1 - p) * 16 + (15 - i)
                                if len(slots) > left:
                                    emit_slot()
                        for i in range(HC - 4, HC):
                            emit_av(i)
                        # stage raw outputs + denominators out of PSUM (the
                        # copies release the pAB WAR for the next block), then
                        # normalize behind the next block (Pool multiply)
                        nc.vector.tensor_copy(uoT[0:64, p, jb], pAB[0:64, 0:512])
                        nc.vector.tensor_copy(
                            uoT[64:128, p, jb], pAB[0:64, 512:1024]
                        )
                        cs = ph2t.tile([33, 512], F32, tag="cs")
                        nc.vector.tensor_copy(cs[0:1, :], pAB[64:65, 0:512])
                        nc.vector.tensor_copy(cs[32:33, :], pAB[64:65, 512:1024])
                        rcs = ph2t.tile([33, 512], F32, tag="rcs")
                        nc.vector.reciprocal(rcs[:], cs[:])
                        cs_dram = csd.tile([2, 1, 512], F32, tag="csd")
                        nc.sync.dma_start(cs_dram[0], rcs[0:1, :])
                        nc.sync.dma_start(cs_dram[1], rcs[32:33, :])
                        cb = ph2t.tile([128, 512], F32, tag="cb")
                        nc.sync.dma_start(
                            cb[0:64, :], cs_dram[0].to_broadcast((64, 512))
                        )
                        nc.sync.dma_start(
                            cb[64:128, :], cs_dram[1].to_broadcast((64, 512))
                        )
                        nc.gpsimd.tensor_tensor(
                            aoT[0:64, p, jb], uoT[0:64, p, jb], cb[0:64, :], MUL
                        )
                        nc.gpsimd.tensor_tensor(
                            aoT[64:128, p, jb], uoT[64:128, p, jb],
                            cb[64:128, :], MUL
                        )
                queue_wo(NB - 1)
                while slots:
                    emit_slot()

    nc.compile()
    return nc


def kernel(
    hidden_states,
    attention_mask,
    position_ids,
    wq,
    wk,
    wv,
    wo,
    _trace=False,
):
    global LAST_EXEC_NS, LAST_TRACE, LAST_RES
    x = np.asarray(hidden_states, np.float32)[0]  # [S, H]
    mask = np.asarray(attention_mask, np.float32)[0]  # [S]
    pos = np.asarray(position_ids)[0].astype(np.float32)  # [S]

    wq_t, s_q = _ternarize(wq)
    wk_t, s_k = _ternarize(wk)
    wv_t, s_v = _ternarize(wv)
    wo_t, s_o = _ternarize(wo)
    s_qk = float(np.float32(s_q) * np.float32(s_k) / np.float32(8.0))
    s_vo = np.float32(s_v) * np.float32(s_o)

    key = ("v8", s_qk)
    if key not in _CACHE:
        _CACHE.clear()
        _CACHE[key] = _build_program(s_qk)
    nc = _CACHE[key]

    # shared inputs
    xt_host = np.ascontiguousarray(
        x.T.reshape(HC, 128, NB, 512).transpose(2, 1, 0, 3)
    ).astype(np.float16)
    inv = (
        1.0 / (10000.0 ** (np.arange(0, D, 2, dtype=np.float32) / np.float32(D)))
    ).astype(np.float32)
    fr = pos[:, None] * inv[None, :]  # [S, 32]
    emb = np.concatenate([fr, fr], axis=1)  # [S, 64]
    cos64 = np.cos(emb).astype(np.float32)
    sin64 = np.sin(emb).astype(np.float32)
    sin64[:, : D // 2] *= -1.0
    cos128 = np.ascontiguousarray(cos64.T).astype(np.float16)
    sin128 = np.ascontiguousarray(sin64.T).astype(np.float16)
    mask_r = np.ascontiguousarray(mask.reshape(HC, 128).T).astype(np.float32)
    # rotate-half block-swap permutation (lhsT), sign lives in sin128
    i32 = np.eye(32, dtype=np.float16)
    z32 = np.zeros((32, 32), dtype=np.float16)
    p64 = np.block([[z32, i32], [i32, z32]]).astype(np.float16)
    prot_host = np.zeros((128, 128), dtype=np.float16)
    prot_host[0:64, 0:64] = p64
    prot_host[64:128, 64:128] = p64

    in_maps = []
    for c in range(NCORES):
        wq_c = np.ascontiguousarray(
            wq_t[c * OC : (c + 1) * OC, :].T.reshape(HC, 128, OC).transpose(1, 0, 2)
        ).astype(np.float16)
        wk_c = wk_t[c * D : (c + 1) * D, :].T  # [H, 64]
        wv_c = (wv_t[c * D : (c + 1) * D, :] * s_vo).T  # fold s_v*s_o into wv
        wkv_c = np.ascontiguousarray(
            np.concatenate([wk_c, wv_c], axis=1).reshape(HC, 128, 128).transpose(1, 0, 2)
        ).astype(np.float16)
        wo_c = np.ascontiguousarray(
            wo_t[:, c * OC : (c + 1) * OC].T.reshape(2, 128, H).transpose(1, 0, 2)
        ).astype(np.float16)
        in_maps.append(
            {
                "xt": xt_host,
                "wq_t": wq_c,
                "wkv_t": wkv_c,
                "wo_t": wo_c,
                "cos_t": cos128,
                "sin_t": sin128,
                "mask_t": mask_r,
                "prot_t": prot_host,
            }
        )

    res = run_bass_kernel_spmd(
        nc, in_maps, core_ids=list(range(NCORES)), trace=bool(_trace)
    )
    LAST_EXEC_NS = res.exec_time_ns
    LAST_TRACE = res.instructions_and_trace[1] if res.instructions_and_trace else None
    LAST_RES = res

    out = res.results[0]["outp"].astype(np.float32)
    for c in range(1, NCORES):
        out = out + res.results[c]["outp"].astype(np.float32)
    return out.reshape(1, S, H).astype(np.float32)
